# revision 2
# baseline (speedup 1.0000x reference)
"""nn_DynamicEdgeWeighter Trainium2 kernel (8 NeuronCores, SPMD).

Strategy (two launches over 8 cores):
  Phase 1 -- node-sharded feature-table build: core c reads
    x_raw[:, :, c*6250:(c+1)*6250, :], computes per-(node, batch) window
    mean/std features, L2-normalizes them (cosine prep) and emits an fp16
    table slice with "pair rows": row j = [fhat(2j) | fhat(2j+1)], each
    128 fp16 = (4 batches x 32 dims), so one 512B row serves 2 nodes and
    row ids fit dma_gather's int16 indices.
  Host: concatenates table slices (25000, 256) fp16.
  Phase 2 -- edge-sharded similarity: core c owns 6250 edges. Members and
    centers are fetched with gpsimd dma_gather (transpose mode -> data
    lands as (batch*dim, member) columns), dotted via fp16 products + a
    PE partition-reduction (block stationary), the even/odd node of each
    pair row resolved by a parity-predicated blend of the two dot planes,
    clipped to [0,1], and summed per edge on-chip.
  Host: assembles per-edge sums, applies mean + per-batch min-max
    normalization and the final W*(1+0.3*ns) scale (O(B*E) scalar work).
"""
from contextlib import ExitStack

import numpy as np

import concourse.bass as bass
import concourse.bacc as bacc
import concourse.mybir as mybir
from concourse import library_config
from concourse.tile import TileContext

F32 = mybir.dt.float32
F16 = mybir.dt.float16
I16 = mybir.dt.int16
U8 = mybir.dt.uint8
AX = mybir.AxisListType
OP = mybir.AluOpType

# problem constants (hardcoded per harness contract)
B, T, N, C = 4, 16, 50000, 16
E, DEG = 50000, 16
LAM = 0.3
NORM_EPS = 1e-8
N_CORES = 8

NODES_PC = N // N_CORES          # 6250
PAIRS_PC = NODES_PC // 2         # 3125
N_PAIRS = N // 2                 # 25000
K = 7168                         # member slots per compute super-call
GC = 896                         # slots per dma_gather call (Q7 scratch limit)
GPS = K // GC                    # 8 gathers per super-call
RW = K // 16                     # 448: quadrant matmul width
VQ = RW // DEG                   # 28 edges per quadrant slice
EDGES_PC = E // N_CORES          # 6250
EPC = K // DEG                   # 448 edges per super-call
CALLS = -(-EDGES_PC // EPC)      # 14
ESLOTS = CALLS * EPC             # 6272
MSLOTS = ESLOTS * DEG            # 100352
GCC = 896                        # center gather chunk
assert ESLOTS % GCC == 0


def _build_phase1():
    P = 128
    nc = bacc.Bacc("TRN2", target_bir_lowering=False, debug=False,
                   num_devices=N_CORES)
    xs = nc.dram_tensor("xs", [B, T, NODES_PC, C], F32, kind="ExternalInput")
    ts = nc.dram_tensor("tslice", [PAIRS_PC, 256], F16, kind="ExternalOutput")
    n_tiles = -(-PAIRS_PC // P)
    SQUARE = mybir.ActivationFunctionType.Square
    SQRT = mybir.ActivationFunctionType.Sqrt
    with TileContext(nc) as tc, ExitStack() as ctx:
        work = ctx.enter_context(tc.tile_pool(name="work", bufs=3))
        stat = ctx.enter_context(tc.tile_pool(name="stat", bufs=3))
        for it in range(n_tiles):
            j0 = it * P
            p = min(P, PAIRS_PC - j0)
            xt = work.tile([P, B, T, 32], F32)
            src = xs.ap()[:, :, 2 * j0: 2 * (j0 + p), :] \
                .rearrange("b t (j u) c -> j b t (u c)", u=2)
            nc.sync.dma_start(out=xt[:p], in_=src)
            xsq = work.tile([P, B, T, 32], F32)
            nc.scalar.activation(xsq[:p], xt[:p], SQUARE)
            sums = stat.tile([P, B, 32], F32)
            nc.vector.tensor_reduce(out=sums[:p], in_=xt[:p].rearrange(
                "j b t m -> j b m t"), axis=AX.X, op=OP.add)
            sumsq = stat.tile([P, B, 32], F32)
            nc.vector.tensor_reduce(out=sumsq[:p], in_=xsq[:p].rearrange(
                "j b t m -> j b m t"), axis=AX.X, op=OP.add)
            mu = stat.tile([P, B, 32], F32)
            nc.vector.tensor_scalar_mul(mu[:p], sums[:p], 1.0 / T)
            ex2 = stat.tile([P, B, 32], F32)
            nc.vector.tensor_scalar_mul(ex2[:p], sumsq[:p], 1.0 / T)
            musq = stat.tile([P, B, 32], F32)
            nc.vector.tensor_mul(musq[:p], mu[:p], mu[:p])
            var = stat.tile([P, B, 32], F32)
            nc.vector.tensor_sub(var[:p], ex2[:p], musq[:p])
            # ||[mu, sd]||^2 over c == sum_c E[x^2] (exact algebraic identity)
            nsq = stat.tile([P, B, 2], F32)
            nc.vector.tensor_reduce(out=nsq[:p], in_=ex2[:p].rearrange(
                "j b (u c) -> j b u c", u=2), axis=AX.X, op=OP.add)
            nrm = stat.tile([P, B, 2], F32)
            nc.scalar.activation(nrm[:p], nsq[:p], SQRT)
            nc.vector.tensor_scalar_max(nrm[:p], nrm[:p], 1e-8)
            rn = stat.tile([P, B, 2], F32)
            nc.vector.reciprocal(rn[:p], nrm[:p])
            sd = stat.tile([P, B, 32], F32)
            nc.scalar.activation(sd[:p], var[:p], SQRT)
            fh = work.tile([P, 2, B, 32], F16)
            rnv = rn[:p].rearrange("j b u -> j u b").unsqueeze(3) \
                .to_broadcast([p, 2, B, 16])
            nc.vector.tensor_mul(
                fh[:p, :, :, 0:16],
                mu[:p].rearrange("j b (u c) -> j u b c", u=2), rnv)
            nc.vector.tensor_mul(
                fh[:p, :, :, 16:32],
                sd[:p].rearrange("j b (u c) -> j u b c", u=2), rnv)
            nc.sync.dma_start(
                out=ts.ap()[j0:j0 + p, :],
                in_=fh[:p].rearrange("j u b c -> j (u b c)"))
    nc.compile()
    return nc


def _build_phase2():
    P = 128
    nc = bacc.Bacc("TRN2", target_bir_lowering=False, debug=False,
                   num_devices=N_CORES, num_swdge_queues=4)
    table = nc.dram_tensor("table", [N_PAIRS, 256], F16, kind="ExternalInput")
    midx = nc.dram_tensor("midx", [128, MSLOTS // 16], I16, kind="ExternalInput")
    cidx = nc.dram_tensor("cidx", [128, ESLOTS // 16], I16, kind="ExternalInput")
    mpar = nc.dram_tensor("mpar", [CALLS, 4, P, RW], U8, kind="ExternalInput")
    cpar = nc.dram_tensor("cpar", [P, ESLOTS], U8, kind="ExternalInput")
    esums = nc.dram_tensor("esums", [P, CALLS * 4 * VQ], F32, kind="ExternalOutput")
    RELU = mybir.ActivationFunctionType.Relu

    with TileContext(nc) as tc, ExitStack() as ctx:
        nc.gpsimd.load_library(library_config.mlp)
        singles = ctx.enter_context(tc.tile_pool(name="singles", bufs=1))
        # stationary: S[p, s] = 1 if p//32 == s%4 (rows = 8 replicas x 4 batches)
        S = singles.tile([P, 32], F16)
        nc.vector.memset(S[:], 0.0)
        for b in range(4):
            nc.vector.memset(S[32 * b:32 * (b + 1), b::4], 1.0)
        midx_sb = singles.tile([128, MSLOTS // 16], I16)
        nc.sync.dma_start(out=midx_sb[:], in_=midx.ap())
        cidx_sb = singles.tile([128, ESLOTS // 16], I16)
        nc.sync.dma_start(out=cidx_sb[:], in_=cidx.ap())
        es_all = singles.tile([P, CALLS * 4 * VQ], F32)
        cmerged = singles.tile([P, ESLOTS], F16)
        mpar_sb = singles.tile([P, CALLS, 4, RW], U8)
        nc.sync.dma_start(out=mpar_sb[:],
                          in_=mpar.ap().rearrange("k r p w -> p k r w"))

        with tc.tile_pool(name="cprep", bufs=1) as cp:
            ncc = ESLOTS // GCC
            cg = cp.tile([P, ncc, 2, GCC], F16)
            for i in range(ncc):
                nc.gpsimd.dma_gather(
                    out_ap=cg[:, i], in_ap=table.ap(),
                    idxs_ap=cidx_sb[:, i * (GCC // 16):(i + 1) * (GCC // 16)],
                    num_idxs=GCC, num_idxs_reg=GCC,
                    elem_size=256, transpose=True, queue_num=0)
            cpar_sb = cp.tile([P, ESLOTS], U8)
            nc.sync.dma_start(out=cpar_sb[:], in_=cpar.ap())
            cpv = cpar_sb[:].rearrange("p (i g) -> p i g", g=GCC)
            nc.vector.tensor_copy(
                out=cmerged[:].rearrange("p (i g) -> p i g", g=GCC),
                in_=cg[:, :, 0, :])
            nc.vector.copy_predicated(
                out=cmerged[:].rearrange("p (i g) -> p i g", g=GCC),
                mask=cpv, data=cg[:, :, 1, :])

        stream = ctx.enter_context(tc.tile_pool(name="stream", bufs=2))
        dots = ctx.enter_context(tc.tile_pool(name="dots", bufs=4))
        psum = ctx.enter_context(tc.tile_pool(name="psum", bufs=4, space="PSUM"))
        for k in range(CALLS):
            g = stream.tile([P, GPS, 2, GC], F16)
            for i in range(GPS):
                col = k * GPS + i
                nc.gpsimd.dma_gather(
                    out_ap=g[:, i], in_ap=table.ap(),
                    idxs_ap=midx_sb[:, col * (GC // 16):(col + 1) * (GC // 16)],
                    num_idxs=GC, num_idxs_reg=GC, elem_size=256, transpose=True,
                    queue_num=0)
            E0 = k * EPC
            cview = cmerged[:, E0:E0 + EPC] \
                .rearrange("p (i e) -> p i e", i=GPS).unsqueeze(3) \
                .to_broadcast([P, GPS, GC // DEG, DEG])
            prodA = stream.tile([P, K], F16)
            nc.vector.tensor_mul(
                prodA[:].rearrange("p (i e d) -> p i e d", i=GPS, d=DEG),
                g[:, :, 0, :].rearrange("p i (e d) -> p i e d", d=DEG), cview)
            prodB = stream.tile([P, K], F16)
            nc.vector.tensor_mul(
                prodB[:].rearrange("p (i e d) -> p i e d", i=GPS, d=DEG),
                g[:, :, 1, :].rearrange("p i (e d) -> p i e d", d=DEG), cview)
            for r in range(4):
                pA = psum.tile([P, RW], F32)
                pB = psum.tile([P, RW], F32)
                for q in range(4):
                    off = (r * 4 + q) * RW
                    nc.tensor.matmul(pA[32 * q:32 * (q + 1), :], S[:],
                                     prodA[:, off:off + RW],
                                     start=True, stop=True,
                                     tile_position=(0, 32 * q))
                    nc.tensor.matmul(pB[32 * q:32 * (q + 1), :], S[:],
                                     prodB[:, off:off + RW],
                                     start=True, stop=True,
                                     tile_position=(0, 32 * q))
                dA = dots.tile([P, RW], F32)
                nc.scalar.activation(dA[:], pA[:], RELU)
                dB = dots.tile([P, RW], F32)
                nc.scalar.activation(dB[:], pB[:], RELU)
                nc.vector.copy_predicated(out=dA[:], mask=mpar_sb[:, k, r, :],
                                          data=dB[:])
                nc.vector.tensor_scalar_min(dA[:], dA[:], 1.0)
                nc.vector.tensor_reduce(
                    out=es_all[:, (k * 4 + r) * VQ:(k * 4 + r + 1) * VQ],
                    in_=dA[:].rearrange("p (v d) -> p v d", d=DEG),
                    axis=AX.X, op=OP.add)
        nc.sync.dma_start(out=esums.ap(), in_=es_all[:])
    nc.compile()
    return nc


class _Runner:
    """Compiled Bass program -> reusable 8-core jitted callable."""

    def __init__(self, nc):
        import jax
        from jax.sharding import Mesh, PartitionSpec
        from jax.experimental.shard_map import shard_map
        from concourse.bass2jax import (_bass_exec_p, partition_id_tensor,
                                        install_neuronx_cc_hook)
        install_neuronx_cc_hook()
        self.jax = jax
        self.PartitionSpec = PartitionSpec
        partition_name = (nc.partition_id_tensor.name
                          if nc.partition_id_tensor else None)
        in_names, out_names, out_avals, zero_outs = [], [], [], []
        for alloc in nc.m.functions[0].allocations:
            if not isinstance(alloc, mybir.MemoryLocationSet):
                continue
            name = alloc.memorylocations[0].name
            if alloc.kind == "ExternalInput":
                if name != partition_name:
                    in_names.append(name)
            elif alloc.kind == "ExternalOutput":
                shape = tuple(alloc.tensor_shape)
                dtype = mybir.dt.np(alloc.dtype)
                out_names.append(name)
                out_avals.append(jax.core.ShapedArray(shape, dtype))
                zero_outs.append(np.zeros(shape, dtype))
        self.in_names = in_names
        self.out_names = out_names
        self.out_avals = out_avals
        self.zero_outs = zero_outs
        n_params = len(in_names)
        n_outs = len(out_avals)
        all_in_names = in_names + out_names
        if partition_name is not None:
            all_in_names.append(partition_name)

        def _body(*args):
            operands = list(args)
            if partition_name is not None:
                operands.append(partition_id_tensor())
            outs = _bass_exec_p.bind(
                *operands,
                out_avals=tuple(out_avals),
                in_names=tuple(all_in_names),
                out_names=tuple(out_names),
                lowering_input_output_aliases=(),
                sim_require_finite=True,
                sim_require_nnan=True,
                nc=nc,
            )
            return tuple(outs)

        devices = jax.devices()[:N_CORES]
        self.mesh = Mesh(np.asarray(devices), ("core",))
        in_specs = (PartitionSpec("core"),) * (n_params + n_outs)
        out_specs = (PartitionSpec("core"),) * n_outs
        self.fn = jax.jit(
            shard_map(_body, mesh=self.mesh, in_specs=in_specs,
                      out_specs=out_specs, check_rep=False),
            keep_unused=True,
        )

    def run(self, concat_inputs):
        """concat_inputs: list of (8*dim0, ...) arrays in in_names order."""
        args = list(concat_inputs)
        for z in self.zero_outs:
            args.append(np.zeros((N_CORES * z.shape[0], *z.shape[1:]), z.dtype))
        outs = self.fn(*args)
        return [np.asarray(o) for o in outs]


_STATE = {}


def _get_runners():
    if "r1" not in _STATE:
        _STATE["r1"] = _Runner(_build_phase1())
        _STATE["r2"] = _Runner(_build_phase2())
    return _STATE["r1"], _STATE["r2"]


def _prep_phase2_host(edge_members, edge_centers):
    """Concatenated (8*dim0, ...) index/parity arrays for phase 2."""
    mem = np.zeros((N_CORES, MSLOTS), dtype=np.int32)
    cen = np.zeros((N_CORES, ESLOTS), dtype=np.int32)
    mem[:, :EDGES_PC * DEG] = edge_members.reshape(N_CORES, EDGES_PC * DEG)
    cen[:, :EDGES_PC] = edge_centers.reshape(N_CORES, EDGES_PC)
    midx = (mem >> 1).astype(np.int16)
    mpar = (mem & 1).astype(np.uint8)
    cidx = (cen >> 1).astype(np.int16)
    cpar = (cen & 1).astype(np.uint8)
    # wrapped-16, replicated x8 along partitions
    midx_w = np.tile(midx.reshape(N_CORES, MSLOTS // 16, 16).transpose(0, 2, 1),
                     (1, 8, 1)).reshape(N_CORES * 128, MSLOTS // 16)
    cidx_w = np.tile(cidx.reshape(N_CORES, ESLOTS // 16, 16).transpose(0, 2, 1),
                     (1, 8, 1)).reshape(N_CORES * 128, ESLOTS // 16)
    # member parity in psum layout (CALLS, 4, 128, RW)
    pm = mpar.reshape(N_CORES, CALLS, 4, 4, 1, RW)
    mpar_dot = np.ascontiguousarray(np.broadcast_to(
        pm, (N_CORES, CALLS, 4, 4, 32, RW))).reshape(
        N_CORES * CALLS, 4, 128, RW)
    cpar_full = np.ascontiguousarray(np.broadcast_to(
        cpar[:, None, :], (N_CORES, 128, ESLOTS))).reshape(
        N_CORES * 128, ESLOTS)
    return midx_w, cidx_w, mpar_dot, cpar_full


def _unpack_esums(es_concat):
    """(8*128, CALLS*4*VQ) -> mean_sim (B, E)."""
    es = es_concat.reshape(N_CORES, 128, CALLS, 4, VQ)
    # es[c, 32q+s, k, r, v] = edgesum(edge c*EDGES_PC + k*EPC + (r*4+q)*VQ + v,
    #                                 batch s%4), s//4 are replicas
    ms = np.zeros((B, E), dtype=np.float32)
    for q in range(4):
        for b in range(4):
            rows = es[:, 32 * q + b]          # (N_CORES, CALLS, 4, VQ)
            el = (np.arange(CALLS)[:, None, None] * EPC
                  + (np.arange(4)[None, :, None] * 4 + q) * VQ
                  + np.arange(VQ)[None, None, :])      # (CALLS, 4, VQ)
            flat_el = el.reshape(-1)
            valid = flat_el < EDGES_PC
            fe = flat_el[valid]
            vals = rows.reshape(N_CORES, -1)[:, valid]  # (N_CORES, nvalid)
            idx = (np.arange(N_CORES)[:, None] * EDGES_PC + fe[None, :]).reshape(-1)
            ms[b, idx] = vals.reshape(-1)
    return ms / DEG


def kernel(x_raw, H, W, edge_members, edge_centers, edge_offsets):
    x_raw = np.ascontiguousarray(np.asarray(x_raw, dtype=np.float32))
    W_arr = np.asarray(W, dtype=np.float32)
    edge_members = np.asarray(edge_members, dtype=np.int32)
    edge_centers = np.asarray(edge_centers, dtype=np.int32)

    r1, r2 = _get_runners()

    # ---- phase 1: feature table ----
    xs_cat = np.ascontiguousarray(
        x_raw.reshape(B, T, N_CORES, NODES_PC, C).transpose(2, 0, 1, 3, 4)
    ).reshape(N_CORES * B, T, NODES_PC, C)
    (ts_cat,) = r1.run([xs_cat])
    table = np.ascontiguousarray(ts_cat)  # (8*3125, 256) = (25000, 256) f16

    # ---- phase 2: gather + cosine sims + edge sums ----
    midx_w, cidx_w, mpar_dot, cpar_full = _prep_phase2_host(
        edge_members, edge_centers)
    table_cat = np.tile(table.reshape(1, N_PAIRS, 256),
                        (N_CORES, 1, 1)).reshape(N_CORES * N_PAIRS, 256)
    ins = []
    for name in r2.in_names:
        ins.append({"table": table_cat, "midx": midx_w, "cidx": cidx_w,
                    "mpar": mpar_dot, "cpar": cpar_full}[name])
    (es_cat,) = r2.run(ins)

    # ---- host finish: mean + min-max normalize + scale ----
    ms = _unpack_esums(es_cat)
    smin = ms.min(axis=1, keepdims=True)
    smax = ms.max(axis=1, keepdims=True)
    ns = (ms - smin) / (smax - smin + NORM_EPS)
    return (W_arr[None, :] * (1.0 + LAM * ns)).astype(np.float32)
